# revision 1
# baseline (speedup 1.0000x reference)
"""Trainium2 Bass kernel for nn_AC_Filter_PreNorm_Net (causal attention + product-network Euler).

Self-contained: accepts FULL inputs, shards batch over 8 NeuronCores, returns FULL output.

Restructured dataflow (validated against reference in numpy, rel err ~6e-3 all-bf16):
  - sigma pre-norm folded into in_proj weights (host)
  - transposed activations: qT/kT [e, L] so scores come out as sT[kv, q]
  - no-max-subtraction softmax (max |score| ~ 13, exp fine in fp32)
  - v augmented with a ones column -> attention output row 64 = softmax denominator
  - out_proj matmul carries the denominator (Wout_aug row-0 selector)
  - normalization deferred until after out_proj, applied once via a DRAM-broadcast
    reciprocal row
  - Euler product tree with host-reordered Wall columns so every level pairs
    (k, k+half) contiguously; "transposes" done as normal matmuls against identity
"""
import sys
sys.path.insert(0, "/opt/trn_rl_repo")
import numpy as np
import concourse.bass as bass
import concourse.tile as tile
import bass_rust
from concourse import mybir
from concourse.bass_utils import run_bass_kernel_spmd

F32 = mybir.dt.float32
BF16 = mybir.dt.bfloat16
AF = mybir.ActivationFunctionType
MULT = mybir.AluOpType.mult
ADD = mybir.AluOpType.add

B, L, D = 16, 2048, 63
E = D + 1            # 64
W1 = 8
F_LEN = 4
DT = 0.01
EPS = 1e-5
NCORES = 8
BPC = B // NCORES    # batches per core = 2
NT = L // 128        # l-tiles per batch = 16
NC4 = 4              # q-chunks of 512


def _split_multiwaits(nc):
    """walrus here rejects >1 sync wait per instruction; hoist extras onto
    preceding same-engine NoOps."""
    n_added = 0
    for fn in nc.m.functions:
        for bb in fn.blocks:
            insts = list(bb.instructions)
            out = []
            changed = False
            for inst in insts:
                si = inst.sync_info
                if si is not None and si.on_wait is not None and len(si.on_wait) > 1:
                    waits = list(si.on_wait)
                    for w in waits[:-1]:
                        nop = mybir.InstNoOp(
                            name=f"{inst.name}-wsp{n_added}", ins=[], outs=[]
                        )
                        n_added += 1
                        nop.engine = inst.engine
                        nop.sync_info = bass_rust.SyncInfo(on_wait=[w], on_update=[])
                        out.append(nop)
                    si.on_wait = [waits[-1]]
                    changed = True
                out.append(inst)
            if changed:
                bb.instructions = out
    return n_added


def _build_nc():
    nc = bass.Bass()
    dp = nc.declare_dram_parameter
    xt_e = dp("xt", [BPC, E, L], BF16, isOutput=False)          # host-pretransposed
    wqkt_e = dp("wqkt", [E, 128], BF16, isOutput=False)         # lhsT: [e_in, q|k out]
    wvt_e = dp("wvt", [E, E], BF16, isOutput=False)             # rhs: [e_in, e_out]
    woutkt_e = dp("woutkt", [E + 1, E], BF16, isOutput=False)   # [65, 64] both lhsT & rhs
    wall_e = dp("wall", [E, D * W1], BF16, isOutput=False)      # [64, 504] tree-ordered
    masks_e = dp("masks", [128, 4 * 512], BF16, isOutput=False)
    ident_e = dp("ident", [128, 128], BF16, isOutput=False)
    srep_e = dp("srep", [128, E], F32, isOutput=False)          # col0=0, col 1+d = s[d]
    out_e = dp("out", [BPC, L, F_LEN * D], F32, isOutput=True)

    with tile.TileContext(nc) as tc:
        with (
            tc.tile_pool(name="consts", bufs=1) as cp,
            tc.tile_pool(name="big", bufs=2) as bp,
            tc.tile_pool(name="chk", bufs=3) as chp,
            tc.tile_pool(name="outp", bufs=4) as op_pool,
            tc.tile_pool(name="ps", bufs=4, space="PSUM") as psP,
        ):
            # ---- constants ----
            wqkt = cp.tile([E, 128], BF16)
            nc.sync.dma_start(out=wqkt[:], in_=wqkt_e[:])
            wvt = cp.tile([E, E], BF16)
            nc.sync.dma_start(out=wvt[:], in_=wvt_e[:])
            woutkt = cp.tile([E + 1, E], BF16)
            nc.sync.dma_start(out=woutkt[:], in_=woutkt_e[:])
            wall = cp.tile([E, D * W1], BF16)
            nc.sync.dma_start(out=wall[:], in_=wall_e[:])
            masks = cp.tile([128, 4 * 512], BF16)
            nc.sync.dma_start(out=masks[:], in_=masks_e[:])
            ident = cp.tile([128, 128], BF16)
            nc.sync.dma_start(out=ident[:], in_=ident_e[:])
            srep = cp.tile([128, E], F32)
            nc.sync.dma_start(out=srep[:], in_=srep_e[:])
            ones_bf = cp.tile([128, E], BF16)
            nc.vector.memset(ones_bf[:], 1.0)

            bstate = {}

            def emit_prologue(b):
                xt = bp.tile([E, L], BF16, tag="xt")
                nc.sync.dma_start(out=xt[:], in_=xt_e[b])
                qT = bp.tile([E, L], BF16, tag="qT")
                kT = bp.tile([E, L], BF16, tag="kT")
                for cp_ in range(2):
                    ps = psP.tile([128, 1024], F32, tag="ps")
                    for u in range(2):
                        c = 2 * cp_ + u
                        nc.tensor.matmul(
                            ps[:, u * 512:(u + 1) * 512], wqkt[:],
                            xt[:, c * 512:(c + 1) * 512], start=True, stop=True)
                    nc.scalar.copy(qT[:, cp_ * 1024:(cp_ + 1) * 1024], ps[0:E, :])
                    nc.vector.tensor_copy(kT[:, cp_ * 1024:(cp_ + 1) * 1024],
                                          ps[64:128, :])
                v_aug = bp.tile([128, NT * (E + 1)], BF16, tag="v_aug")
                v_aug_v = v_aug[:].rearrange("p (n e1) -> p n e1", e1=E + 1)
                nc.vector.memset(v_aug_v[:, :, E:E + 1], 1.0)
                ps = psP.tile([128, 1024], F32, tag="ps")
                for lt in range(NT):
                    nc.tensor.matmul(
                        ps[:, lt * E:(lt + 1) * E],
                        xt[:, lt * 128:(lt + 1) * 128], wvt[:],
                        start=True, stop=True)
                nc.vector.tensor_copy(v_aug_v[:, :, 0:E],
                                      ps[:].rearrange("p (j e) -> p j e", e=E))
                bstate[b] = {"qT": qT, "kT": kT, "v_aug": v_aug}

            def emit_scores_exp(b, c):
                st = bstate[b]
                qT, kT = st["qT"], st["kT"]
                nki = 4 * c + 4
                exps = chp.tile([128, NT * 512], BF16, tag="exps")
                ki = 0
                while ki < nki:
                    g = min(2, nki - ki)
                    ps = psP.tile([128, 1024], F32, tag="ps")
                    for j in range(g):
                        nc.tensor.matmul(
                            ps[:, j * 512:(j + 1) * 512],
                            kT[:, (ki + j) * 128:(ki + j + 1) * 128],
                            qT[:, c * 512:(c + 1) * 512], start=True, stop=True)
                    nc.scalar.activation(exps[:, ki * 512:(ki + g) * 512],
                                         ps[:, 0:g * 512], AF.Exp)
                    ki += g
                for off in range(4):
                    kb = 4 * c + off
                    nc.gpsimd.tensor_tensor(
                        exps[:, kb * 512:(kb + 1) * 512],
                        exps[:, kb * 512:(kb + 1) * 512],
                        masks[:, off * 512:(off + 1) * 512], MULT)
                bstate[(b, c, "exps")] = exps

            def emit_att_tail(b, c):
                st = bstate[b]
                v_aug = st["v_aug"]
                nki = 4 * c + 4
                exps = bstate.pop((b, c, "exps"))
                po = psP.tile([128, 1024], F32, tag="ps")
                pov = po[0:E + 1, 0:512]
                for ki in range(nki):
                    nc.tensor.matmul(
                        pov, v_aug[:, ki * (E + 1):(ki + 1) * (E + 1)],
                        exps[:, ki * 512:(ki + 1) * 512],
                        start=(ki == 0), stop=(ki == nki - 1))
                o_un = chp.tile([E + 1, 512], BF16, tag="o_un")
                nc.scalar.copy(o_un[:], pov)
                # out_proj + sl-init MMs share one psum tile
                ps2 = psP.tile([128, 1024], F32, tag="ps")
                nc.tensor.matmul(ps2[0:E, 0:512], woutkt[:], o_un[:],
                                 start=True, stop=True)
                stu = chp.tile([E, 512], BF16, tag="stu")
                nc.scalar.copy(stu[:], ps2[0:E, 0:512])
                ps3 = psP.tile([128, 1024], F32, tag="ps")
                for j in range(4):
                    nc.tensor.matmul(
                        ps3[:, j * E:(j + 1) * E],
                        o_un[:, j * 128:(j + 1) * 128], woutkt[:],
                        start=True, stop=True)
                # on-chip reciprocal spread/broadcast (no DRAM roundtrip):
                # 1) spread denom row -> [128, 4] via K=1 matmuls
                pd = psP.tile([128, 1024], F32, tag="ps")
                for j in range(4):
                    nc.tensor.matmul(pd[:, j:j + 1],
                                     o_un[64:65, j * 128:(j + 1) * 128],
                                     ones_bf[64:65, 0:1], start=True, stop=True)
                rden_sp = chp.tile([128, 4], BF16, tag="rden_sp")
                with nc.allow_low_precision(reason="rden in bf16 is fine (0.4% on a softmax denom)"):
                    nc.vector.reciprocal(rden_sp[:], pd[:, 0:4])
                # 2) rebuild rden as a row [1, 512] via identity matmuls
                for j in range(4):
                    nc.tensor.matmul(pd[0:1, 512 + j * 128:512 + (j + 1) * 128],
                                     rden_sp[:, j:j + 1], ident[:],
                                     start=True, stop=True)
                rrow = chp.tile([1, 512], BF16, tag="rrow")
                nc.scalar.copy(rrow[:], pd[0:1, 512:1024])
                # 3) broadcast to [64, 512] via K=1 matmul, then normalize
                pbc = psP.tile([128, 1024], F32, tag="ps")
                nc.tensor.matmul(pbc[0:E, 0:512], ones_bf[0:1, 0:E], rrow[:],
                                 start=True, stop=True)
                stateT = chp.tile([E, 512], BF16, tag="stateT")
                nc.vector.tensor_tensor(stateT[:], stu[:], pbc[0:E, 0:512], MULT)
                state_l = chp.tile([128, 4 * E], F32, tag="state_l")
                nc.vector.tensor_tensor(
                    state_l[:].rearrange("p (j e) -> p j e", e=E),
                    ps3[:, 0:4 * E].rearrange("p (j e) -> p j e", e=E),
                    rden_sp[:, :, None].to_broadcast([128, 4, E]), MULT)
                bstate[(b, c)] = (stateT, state_l)

            def emit_eul_chunk(b, c):
                stateT, state_l = bstate[(b, c)]
                t3 = chp.tile([128, 4 * E], BF16, tag="t3")
                t3_v = t3[:].rearrange("p (n e) -> p n e", e=E)
                nc.vector.memset(t3_v[:, :, 0:1], 0.0)
                for t in range(F_LEN):
                    for g in range(2):   # 2-l-tile groups within the chunk
                        ph = psP.tile([128, 1024], F32, tag="ps")
                        for u in range(2):
                            lt = 2 * g + u
                            nc.tensor.matmul(
                                ph[:, u * 512:u * 512 + 504],
                                stateT[:, lt * 128:(lt + 1) * 128], wall[:],
                                start=True, stop=True)
                        ph_v = ph[:].rearrange("p (u k) -> p u k", k=512)
                        ph_jd = ph_v[:, :, 0:504].rearrange(
                            "p u (j d) -> p u d j", d=D)
                        nc.vector.tensor_reduce(
                            t3_v[:, 2 * g:2 * g + 2, 1:E],
                            ph_jd, mybir.AxisListType.X, MULT)
                    nc.vector.scalar_tensor_tensor(
                        state_l[:], t3[:], DT, state_l[:], MULT, ADD)
                    outbuf = op_pool.tile([128, 4 * E], F32, tag="outbuf")
                    nc.gpsimd.tensor_tensor(
                        outbuf[:].rearrange("p (n e) -> p n e", e=E),
                        state_l[:].rearrange("p (n e) -> p n e", e=E),
                        srep[:, None, :].to_broadcast([128, 4, E]), MULT)
                    pvt = psP.tile([128, 1024], F32, tag="ps")
                    for j in range(4):
                        nc.tensor.matmul(
                            pvt[0:E, j * 128:(j + 1) * 128],
                            t3[:, j * E:(j + 1) * E], ident[:],
                            start=True, stop=True)
                    nc.vector.scalar_tensor_tensor(
                        stateT[:], pvt[0:E, 0:512], DT, stateT[:], MULT, ADD)
                    nc.scalar.dma_start(
                        out=bass.AP(tensor=out_e,
                                    offset=b * L * F_LEN * D + c * 512 * F_LEN * D + t * D,
                                    ap=[[F_LEN * D, 128], [128 * F_LEN * D, 4],
                                        [1, D]]),
                        in_=outbuf[:].rearrange("p (n e) -> p n e", e=E)[:, :, 1:E])

            # ---- software pipeline: scoresExp runs one chunk ahead ----
            chunks = [(b, c) for b in range(BPC) for c in range(NC4)]
            emit_prologue(0)
            emit_scores_exp(*chunks[0])
            for i, (b, c) in enumerate(chunks):
                nxt = chunks[i + 1] if i + 1 < len(chunks) else None
                if nxt is not None:
                    if nxt[1] == 0:
                        emit_prologue(nxt[0])
                    emit_scores_exp(*nxt)
                emit_att_tail(b, c)
                emit_eul_chunk(b, c)

    _split_multiwaits(nc)
    return nc


_NC_CACHE = None


def _get_nc():
    global _NC_CACHE
    if _NC_CACHE is None:
        _NC_CACHE = _build_nc()
    return _NC_CACHE


def kernel(t, inputs, in_proj_w, in_proj_b, out_proj_w, out_proj_b,
           Wg, Mg, bg, sigma):
    inputs = np.asarray(inputs, np.float32)
    in_proj_w = np.asarray(in_proj_w, np.float32)
    in_proj_b = np.asarray(in_proj_b, np.float32)
    out_proj_w = np.asarray(out_proj_w, np.float32)
    out_proj_b = np.asarray(out_proj_b, np.float32)
    Wg = np.asarray(Wg, np.float32)
    Mg = np.asarray(Mg, np.float32)
    bg = np.asarray(bg, np.float32)
    sigma = np.asarray(sigma, np.float32)
    bf = mybir.dt.np(BF16)

    # ---- host-side weight prep ----
    s = sigma + EPS
    inv_s_aug = np.concatenate([[1.0], 1.0 / s]).astype(np.float32)
    Win_f = in_proj_w * inv_s_aug[None, :]
    scale = 1.0 / np.sqrt(np.float32(E))
    Wq = Win_f[0:E] * scale           # fold score scale into q projection
    Wk = Win_f[E:2 * E]
    Wv = Win_f[2 * E:3 * E]
    # (in_proj_b / out_proj_b are zeros in this model; asserted cheaply)
    assert np.all(in_proj_b == 0) and np.all(out_proj_b == 0)

    wqkt = np.concatenate([Wq, Wk], axis=0).T.astype(bf)       # [64, 128]
    wvt = Wv.T.astype(bf)                                      # [64, 64]
    Wout_aug = np.zeros((E, E + 1), np.float32)
    Wout_aug[0, E] = 1.0
    Wout_aug[1:, 0:E] = out_proj_w[1:, :]
    woutkt = Wout_aug.T.astype(bf)                             # [65, 64]

    Wgm = Wg * Mg
    Wall = np.zeros((E, D * W1), np.float32)
    for j in range(W1):
        Wall[:, j * D:(j + 1) * D] = Wgm[:, j, :].T
        Wall[0, j * D:(j + 1) * D] += bg[:, j]
    wall = Wall.astype(bf)

    masks = np.zeros((128, 4 * 512), np.float32)
    kv = np.arange(128)[:, None]
    q = np.arange(512)[None, :]
    for off in range(4):
        masks[:, off * 512:(off + 1) * 512] = (off * 128 + kv <= q)
    masks = masks.astype(bf)
    ident = np.eye(128).astype(bf)
    srep = np.zeros((128, E), np.float32)
    srep[:, 1:] = s[None, :]

    xt_all = np.ascontiguousarray(
        inputs.reshape(NCORES, BPC, L, E).transpose(0, 1, 3, 2)).astype(bf)

    in_maps = []
    for i in range(NCORES):
        in_maps.append({
            "xt": xt_all[i], "wqkt": wqkt, "wvt": wvt, "woutkt": woutkt,
            "wall": wall, "masks": masks, "ident": ident, "srep": srep,
        })

    nc = _get_nc()
    res = run_bass_kernel_spmd(nc, in_maps, core_ids=list(range(NCORES)))
    global LAST_RESULTS
    LAST_RESULTS = res
    out = np.concatenate([res.results[i]["out"] for i in range(NCORES)], axis=0)
    return np.ascontiguousarray(out.astype(np.float32))


LAST_RESULTS = None



# revision 10
# speedup vs baseline: 1.3729x; 1.3729x over previous
"""Trainium2 Bass kernel v2 for nn_AC_Filter_PreNorm_Net (causal attention +
product-network Euler).

Self-contained: accepts FULL inputs, shards batch over 8 NeuronCores, returns
FULL output.

Dataflow (numpy-validated, rel err 6.7e-3 all-bf16):
  - sigma pre-norm folded into in_proj weights (host)
  - out_proj fused into the V projection: u = x @ (Wout Wv)^T, Wu row0 zeroed
  - physical-units state: sigma folded into Wu rows / wall columns; the
    Euler state IS the output (no denorm multiply)
  - softmax normalization via PE transpose: attention output transposed to
    L-major, so the denominator is a column -> native free-dim broadcast
  - Euler in E-major: h = wall_g^T @ stateT per group (wall fixed weights),
    8-factor product as a pairwise bf16 tree (m01 DVE, m23 Pool, rest DVE),
    state updated in place; DT*s_d folded into wall factor-0 columns
  - two batches interleaved at step granularity to keep the PE queue dense
    (p-state ramp: continuously-busy PE runs 2.4GHz vs 1.2GHz)
"""
import sys
sys.path.insert(0, "/opt/trn_rl_repo")
import numpy as np
import concourse.bass as bass
import concourse.tile as tile
import bass_rust
from concourse import mybir
from concourse.bass_utils import run_bass_kernel_spmd

F32 = mybir.dt.float32
BF16 = mybir.dt.bfloat16
AF = mybir.ActivationFunctionType
MULT = mybir.AluOpType.mult
ADD = mybir.AluOpType.add

B, L, D = 16, 2048, 63
E = D + 1            # 64
W1 = 8
F_LEN = 4
DT = 0.01
EPS = 1e-5
NCORES = 8
BPC = B // NCORES    # batches per core = 2
NT = L // 128        # l-tiles per batch = 16
NC4 = 4              # q-chunks of 512


def _split_multiwaits(nc):
    """walrus rejects >1 sync wait per instruction; hoist extras onto
    preceding same-engine NoOps."""
    n_added = 0
    for fn in nc.m.functions:
        for bb in fn.blocks:
            insts = list(bb.instructions)
            out = []
            changed = False
            for inst in insts:
                si = inst.sync_info
                if si is not None and si.on_wait is not None and len(si.on_wait) > 1:
                    waits = list(si.on_wait)
                    for w in waits[:-1]:
                        nop = mybir.InstNoOp(
                            name=f"{inst.name}-wsp{n_added}", ins=[], outs=[]
                        )
                        n_added += 1
                        nop.engine = inst.engine
                        nop.sync_info = bass_rust.SyncInfo(on_wait=[w], on_update=[])
                        out.append(nop)
                    si.on_wait = [waits[-1]]
                    changed = True
                out.append(inst)
            if changed:
                bb.instructions = out
    return n_added


def _build_nc():
    nc = bass.Bass()
    dp = nc.declare_dram_parameter
    xt_e = dp("xt", [BPC, E, L], BF16, isOutput=False)       # host-pretransposed
    wqkt_e = dp("wqkt", [E, 128], BF16, isOutput=False)      # lhsT: [e_in, q|k]
    wut_e = dp("wut", [E, E], BF16, isOutput=False)          # rhs: [e_in, e_out]
    wall_e = dp("wall", [E, 4 * 128], BF16, isOutput=False)  # grouped, DT*s folded
    masks_e = dp("masks", [128, 4 * 512], BF16, isOutput=False)
    ident_e = dp("ident", [128, 128], BF16, isOutput=False)
    out_e = dp("out", [BPC, L, F_LEN * D], F32, isOutput=True)

    with tile.TileContext(nc) as tc:
        with (
            tc.tile_pool(name="consts", bufs=1) as cp,
            tc.tile_pool(name="big", bufs=2) as bp,
            tc.tile_pool(name="chk", bufs=2) as chp,
            tc.tile_pool(name="outp", bufs=2) as op_pool,
            tc.tile_pool(name="ps", bufs=1, space="PSUM") as psP,
        ):
            # ---- constants ----
            wqkt = cp.tile([E, 128], BF16)
            nc.sync.dma_start(out=wqkt[:], in_=wqkt_e[:])
            wut = cp.tile([E, E], BF16)
            nc.sync.dma_start(out=wut[:], in_=wut_e[:])
            wall = cp.tile([E, 4 * 128], BF16)
            nc.sync.dma_start(out=wall[:], in_=wall_e[:])
            masks = cp.tile([128, 4 * 512], BF16)
            nc.sync.dma_start(out=masks[:], in_=masks_e[:])
            ident = cp.tile([128, 128], BF16)
            nc.sync.dma_start(out=ident[:], in_=ident_e[:])

            st = {}   # persistent per-batch tiles

            # ================= attention thunk lists =================
            def attn_thunks(b, c):
                """List of closures emitting attention for (b, c), in
                queue-safe order."""
                ops = []
                nki = 4 * c + 4
                npair = nki // 2

                if c == 0:
                    def ldx(b=b):
                        xt = bp.tile([E, L], BF16, tag="xt")
                        nc.sync.dma_start(out=xt[:], in_=xt_e[b])
                        qT = bp.tile([E, L], BF16, tag="qT")
                        kT = bp.tile([E, L], BF16, tag="kT")
                        u_aug = bp.tile([128, NT * (E + 1)], BF16, tag="u_aug")
                        st[b] = {"xt": xt, "qT": qT, "kT": kT, "u_aug": u_aug}
                    ops.append(ldx)

                    def qk(cp_, b=b):
                        s_ = st[b]
                        ps = psP.tile([128, 1024], F32, tag="sc")
                        for u in range(2):
                            ch = 2 * cp_ + u
                            nc.tensor.matmul(
                                ps[:, u * 512:(u + 1) * 512], wqkt[:],
                                s_["xt"][:, ch * 512:(ch + 1) * 512],
                                start=True, stop=True)
                        nc.scalar.copy(
                            s_["qT"][:, cp_ * 1024:(cp_ + 1) * 1024], ps[0:E, :])
                        nc.vector.tensor_copy(
                            s_["kT"][:, cp_ * 1024:(cp_ + 1) * 1024], ps[64:128, :])
                    ops.append(lambda b=b: qk(0, b))
                    ops.append(lambda b=b: qk(1, b))

                    def uproj(b=b):
                        s_ = st[b]
                        ps = psP.tile([128, 1024], F32, tag="sc")
                        for lt in range(NT):
                            nc.tensor.matmul(
                                ps[:, lt * 64:(lt + 1) * 64],
                                s_["xt"][:, lt * 128:(lt + 1) * 128], wut[:],
                                start=True, stop=True)
                        ua = s_["u_aug"][:].rearrange("p (n e1) -> p n e1", e1=E + 1)
                        nc.vector.memset(ua[:, :, E:E + 1], 1.0)
                        nc.scalar.copy(
                            ua[:, :, 0:E],
                            ps[:].rearrange("p (n e) -> p n e", e=E))
                    ops.append(uproj)

                exps_tiles = {}

                def scpair(j, b=b, c=c, npair=npair):
                    s_ = st[b]
                    ps = psP.tile([128, 1024], F32, tag="sc")
                    for u in range(2):
                        ki = 2 * j + u
                        nc.tensor.matmul(
                            ps[:, u * 512:(u + 1) * 512],
                            s_["kT"][:, ki * 128:(ki + 1) * 128],
                            s_["qT"][:, c * 512:(c + 1) * 512],
                            start=True, stop=True)
                    exps = chp.tile([128, 1024], BF16, tag="exps", bufs=4)
                    nc.scalar.activation(exps[:], ps[:], AF.Exp)
                    off = j - (npair - 2)   # mask the last 2 (diagonal) pairs
                    if off >= 0:
                        # gpsimd: SBUF-only bf16 elementwise, keeps DVE free
                        nc.gpsimd.tensor_tensor(
                            exps[:], exps[:],
                            masks[:, off * 1024:(off + 1) * 1024], MULT)
                    exps_tiles[j] = exps

                def av(ki, b=b, c=c, nki=nki):
                    if ki == 0:
                        pov = psP.tile([65, 512], F32, tag="pt")
                        st[(b, c, "pov")] = pov
                    pov = st[(b, c, "pov")]
                    exps = exps_tiles[ki // 2]
                    eh = exps[:, (ki % 2) * 512:(ki % 2 + 1) * 512]
                    ua = st[b]["u_aug"][:].rearrange(
                        "p (n e1) -> p n e1", e1=E + 1)
                    nc.tensor.matmul(
                        pov[:], ua[:, ki, :], eh,
                        start=(ki == 0), stop=(ki == nki - 1))
                    if ki == nki - 1:
                        exps_tiles.clear()

                # interleave: scores run ~1 pair ahead of av
                sq = list(range(npair))
                aq = list(range(nki))
                while sq or aq:
                    if sq:
                        j = sq.pop(0)
                        ops.append(lambda j=j: scpair(j))
                    done_pairs = npair - len(sq)
                    for _ in range(2):
                        if aq and (not sq or aq[0] <= 2 * done_pairs - 3):
                            ki = aq.pop(0)
                            ops.append(lambda ki=ki: av(ki))

                def tail(b=b, c=c):
                    pov = st.pop((b, c, "pov"))
                    o_un = chp.tile([65, 512], BF16, tag="o_un")
                    nc.scalar.copy(o_un[:], pov[:])
                    # stride 66 keeps each PSUM transpose write 4B-aligned
                    tr = psP.tile([128, 4 * 66], BF16, tag="pt")
                    tr_v = tr[:].rearrange("p (n e1) -> p n e1", e1=66)
                    for j in range(4):
                        nc.tensor.transpose(
                            tr_v[:, j, 0:65],
                            o_un[:, j * 128:(j + 1) * 128],
                            ident[0:65, 0:65])
                    rden = chp.tile([128, 4], BF16, tag="rden")
                    with nc.allow_low_precision(reason="bf16 recip of softmax denom, 0.4%"):
                        nc.vector.reciprocal(rden[:], tr_v[:, :, 64])
                    # state_l columns PERMUTED: col p = state e=p+1 (p<63), col 63 = ones
                    state_l = chp.tile([128, 4 * E], BF16, tag="state_l")
                    sl = state_l[:].rearrange("p (n e) -> p n e", e=E)
                    nc.vector.memset(sl[:, :, D:E], 1.0)
                    nc.vector.tensor_tensor(
                        sl[:, :, 0:D], tr_v[:, :, 1:E],
                        rden[:, :, None].to_broadcast([128, 4, D]), MULT)
                    stT_ps = psP.tile([64, 512], BF16, tag="pt")
                    for j in range(4):
                        nc.tensor.transpose(
                            stT_ps[:, j * 128:(j + 1) * 128],
                            sl[:, j, :], ident[:])
                    stateT = chp.tile([E, 512], BF16, tag="stateT")
                    nc.vector.tensor_copy(stateT[:], stT_ps[:])
                    st[(b, c, "stateT")] = stateT
                ops.append(tail)
                return ops

            # ================= euler emission =================
            def euler_h(b, c, t):
                stateT = st[(b, c, "stateT")]
                hA = psP.tile([128, 1024], F32, tag="hA")
                hB = psP.tile([128, 1024], F32, tag="hB")
                for g in range(4):
                    ps = hA if g < 2 else hB
                    nc.tensor.matmul(
                        ps[:, (g % 2) * 512:(g % 2) * 512 + 512],
                        wall[:, g * 128:(g + 1) * 128], stateT[:],
                        start=True, stop=True)
                st[(b, "hA")] = hA
                st[(b, "hB")] = hB

            def euler_prodA(b):
                hA = st[(b, "hA")]
                hB = st[(b, "hB")]
                # DVE reads only ONE psum operand per instruction -> use a
                # pair-axis reduce (g stride 512) instead of tensor_tensor
                m01 = chp.tile([128, 512], BF16, tag="m01")
                nc.vector.tensor_reduce(
                    m01[:], hA[:].rearrange("p (g l) -> p l g", l=512),
                    mybir.AxisListType.X, MULT)
                m23 = chp.tile([128, 512], BF16, tag="m23")
                nc.vector.tensor_reduce(
                    m23[:], hB[:].rearrange("p (g l) -> p l g", l=512),
                    mybir.AxisListType.X, MULT)
                st[(b, "m01")] = m01
                st[(b, "m23")] = m23

            def euler_prodB(b, c):
                stateT = st[(b, c, "stateT")]
                m01 = st.pop((b, "m01"))
                m23 = st.pop((b, "m23"))
                st.pop((b, "hA"))
                st.pop((b, "hB"))
                mm = chp.tile([128, 512], BF16, tag="mm")
                nc.vector.tensor_tensor(mm[:], m01[:], m23[:], MULT)
                # aligned cross-base copy (64->0); DVE requires equal bases for tt
                msh = chp.tile([63, 512], BF16, tag="msh")
                nc.vector.tensor_copy(msh[:], mm[64:127, :])
                vf = chp.tile([63, 512], BF16, tag="vf")
                nc.vector.tensor_tensor(vf[:], mm[0:63, :], msh[:], MULT)
                nc.vector.tensor_tensor(
                    stateT[0:D, :], stateT[0:D, :], vf[:], ADD)

            def euler_outT(b, c, t):
                stateT = st[(b, c, "stateT")]
                ot = psP.tile([128, 256], F32, tag="ot")
                for j in range(4):
                    nc.tensor.matmul(
                        ot[:, j * 64:(j + 1) * 64],
                        stateT[:, j * 128:(j + 1) * 128],
                        ident[0:64, 0:64], start=True, stop=True)
                ot_v = ot[:].rearrange("p (n e) -> p n e", e=E)
                outbuf = st[(b, "outbuf")]
                ob = outbuf[:].rearrange("p (n f) -> p n f", f=F_LEN * D)
                dst = ob[:, :, t * D:(t + 1) * D]
                nc.scalar.copy(dst, ot_v[:, :, 0:D])   # permuted state: d at col d

            def euler_dma(b, c):
                outbuf = st[(b, "outbuf")]
                ob = outbuf[:].rearrange("p (n f) -> p n f", f=F_LEN * D)
                nc.sync.dma_start(
                    out=bass.AP(
                        tensor=out_e,
                        offset=b * L * F_LEN * D + c * 512 * F_LEN * D,
                        ap=[[F_LEN * D, 128], [128 * F_LEN * D, 4],
                            [1, F_LEN * D]]),
                    in_=ob[:, :, :])

            # ================= schedule =================
            def drain(q, quota):
                n = 0
                while q and n < quota:
                    q.pop(0)()
                    n += 1

            # startup: full attention for chunk 0, both batches
            for b in range(BPC):
                for op in attn_thunks(b, 0):
                    op()
                st[(b, "outbuf")] = op_pool.tile(
                    [128, 4 * F_LEN * D], F32, tag="outbuf", name="outbuf")

            A = {0: [], 1: []}
            for c in range(NC4):
                if c + 1 < NC4:
                    A[0] = attn_thunks(0, c + 1)
                    A[1] = attn_thunks(1, c + 1)
                for t in range(F_LEN):
                    for b in range(BPC):
                        if t > 0:
                            euler_outT(b, c, t - 1)
                        euler_h(b, c, t)
                        euler_prodA(b)
                    # pace attention for next chunk: b0 in slots 0-1, b1 in 2-3
                    src = A[0] if t < 2 else A[1]
                    calls_left = 2 * (2 - t % 2)
                    drain(src, (len(src) + calls_left - 1) // calls_left)
                    for b in range(BPC):
                        euler_prodB(b, c)
                    drain(src, (len(src) + calls_left - 2) // (calls_left - 1))
                    if t in (1, 3):
                        drain(src, len(src))   # finish stream in its last slot
                for b in range(BPC):
                    euler_outT(b, c, F_LEN - 1)
                    euler_dma(b, c)
                    st.pop((b, c, "stateT"))
                    st[(b, "outbuf")] = op_pool.tile(
                        [128, 4 * F_LEN * D], F32, tag="outbuf", name="outbuf")

    _split_multiwaits(nc)
    return nc


_NC_CACHE = None


def _get_nc():
    global _NC_CACHE
    if _NC_CACHE is None:
        _NC_CACHE = _build_nc()
    return _NC_CACHE


def kernel(t, inputs, in_proj_w, in_proj_b, out_proj_w, out_proj_b,
           Wg, Mg, bg, sigma):
    inputs = np.asarray(inputs, np.float32)
    in_proj_w = np.asarray(in_proj_w, np.float32)
    in_proj_b = np.asarray(in_proj_b, np.float32)
    out_proj_w = np.asarray(out_proj_w, np.float32)
    out_proj_b = np.asarray(out_proj_b, np.float32)
    Wg = np.asarray(Wg, np.float32)
    Mg = np.asarray(Mg, np.float32)
    bg = np.asarray(bg, np.float32)
    sigma = np.asarray(sigma, np.float32)
    bf = mybir.dt.np(BF16)

    # ---- host-side weight prep ----
    s = sigma + EPS
    inv_s_aug = np.concatenate([[1.0], 1.0 / s]).astype(np.float32)
    Win_f = in_proj_w * inv_s_aug[None, :]
    scale = 1.0 / np.sqrt(np.float32(E))
    Wq = Win_f[0:E] * scale
    Wk = Win_f[E:2 * E]
    Wv = Win_f[2 * E:3 * E]
    assert np.all(in_proj_b == 0) and np.all(out_proj_b == 0)

    # fused V*out_proj with physical-units fold
    Wu = out_proj_w @ Wv
    Wu[0, :] = 0.0
    Wu[1:, :] = Wu[1:, :] * s[:, None]

    wqkt = np.concatenate([Wq, Wk], axis=0).T.astype(bf)       # [64, 128]
    wut = Wu.T.astype(bf)                                      # [64, 64]

    # wall grouped: group g is [64, 128]: col d <-> factor 2g ch d, col 64+d
    # <-> factor 2g+1 ch d; cols 63/127 dummy-zero. Rows permuted to the
    # stateT layout (states e=1..63 at partitions 0..62, ones-row at 63).
    Wgm = Wg * Mg
    wall = np.zeros((E, 4 * 128), np.float32)
    for g in range(4):
        for f in range(2):
            j = 2 * g + f
            col = g * 128 + f * 64 + np.arange(D)
            wall[D, col] = Wgm[:, j, 0] + bg[:, j]            # ones-row coeff
            wall[0:D, col] = (Wgm[:, j, 1:] / s[None, :]).T   # states rows
    wall[:, 0:63] *= DT * s[None, :]   # fold DT and s_d into factor 0
    wall = wall.astype(bf)

    masks = np.zeros((128, 4 * 512), np.float32)
    kv = np.arange(128)[:, None]
    q = np.arange(512)[None, :]
    for off in range(4):
        masks[:, off * 512:(off + 1) * 512] = (off * 128 + kv <= q)
    masks = masks.astype(bf)
    ident = np.eye(128).astype(bf)

    xt_all = np.ascontiguousarray(
        inputs.reshape(NCORES, BPC, L, E).transpose(0, 1, 3, 2)).astype(bf)

    in_maps = []
    for i in range(NCORES):
        in_maps.append({
            "xt": xt_all[i], "wqkt": wqkt, "wut": wut,
            "wall": wall, "masks": masks, "ident": ident,
        })

    nc = _get_nc()
    res = run_bass_kernel_spmd(nc, in_maps, core_ids=list(range(NCORES)))
    global LAST_RESULTS
    LAST_RESULTS = res
    out = np.concatenate([res.results[i]["out"] for i in range(NCORES)], axis=0)
    return np.ascontiguousarray(out.astype(np.float32))


LAST_RESULTS = None
